# revision 29
# baseline (speedup 1.0000x reference)
"""Causal self-attention Trainium2 Bass kernel.

Problem: B=4, T=2048, DIM=1024, H=16 heads, head_dim=64 (fp32).
  qkv = x @ w_qkv.T ; per-head causal softmax(q k^T / 8) v ; out @ w_out.T

Sharding (8 cores): core c -> (batch b = c//2, head-group g = c%2 of 8 heads).
Each core computes a partial output y_partial = attn_out_g @ w_out[:, g]^T
for its batch; host sums the two head-group partials per batch.

Device layout (per core):
  xt      [1024, 2048] bf16 : x[b]^T (dim-major)           -- host-transposed
  wqkvt   [1024, 1536] bf16 : [Wq|Wk|Wv]^T slice           -- host-transposed
  woutt   [ 512, 1024] bf16 : w_out[:, g]^T                -- host-transposed
  bandm   [ 128,  256] bf16 : keep-triangle (t>=k), two copies side by side
  blkones [   2,  128] bf16 : rows select head A/B cols (recip broadcast)
  yt      [1024, 2048] f32  : partial output, transposed

Structure: the whole kernel is ONE globally interleaved instruction stream.
Scores for 2 heads are row-packed (tile_position 0/64) so each (head-pair,
ktile) is one 512-col PE slot; exp runs on ScalarE; P@V accumulates per-head
output plus a ones-column denominator row.  Because exp (~1.1us/tile) is
slower than the PE slots that feed/consume it, attention alone would stall
the in-order PE queue.  So:
  - P@V emission lags its (scores, exp) item by 3 items, and the softmax
    division is split (DVE reciprocal part / PE-broadcast part) and woven
    between pv flushes so the PE never sits on a not-yet-ready dependency.
  - QKV-projection groups of the NEXT chunk and output-projection groups of
    the PREVIOUS chunk are emitted as fillers between attention items, so
    both PE and ScalarE stay busy through the whole kernel.
Causal masking: only the 4 diagonal ktiles per (head-pair, chunk) need it.
Each diagonal index j has dedicated pre-zeroed e-buffers: exp is only run on
the live columns [128j:512) of each head half (strided 2-group ACT), the
partition-dependent 128-col band is masked with one [128,(2,128)] DVE
multiply, and the fully-killed rectangle stays zero from a one-time memset.
The denominator reciprocal is broadcast across partitions with a tiny
(blkones) matmul instead of a DRAM round-trip.  Stage-3 output tiles are
DMAed PSUM->DRAM directly.
"""

import contextlib
from collections import deque

import numpy as np
import ml_dtypes

import concourse.bass as bass
import concourse.mybir as mybir
import concourse.tile as tile
from concourse import bacc
from concourse.bass_utils import run_bass_kernel_spmd

B, T, DIM = 4, 2048, 1024
NUM_HEADS, HEAD_DIM = 16, 64
INNER = NUM_HEADS * HEAD_DIM
SCALE = HEAD_DIM ** -0.5

N_CORES = 8
HEADS_PER_CORE = 8
HG = HEADS_PER_CORE * HEAD_DIM  # 512 = inner slice per core
NCH = T // 512                  # 4 token chunks
KT_PER_CH = 4                   # 128-ktok tiles per 512 chunk
PV_LAG = 4                      # items between (scores,exp) and its P@V

F32 = mybir.dt.float32
BF16 = mybir.dt.bfloat16


def build_bass():
    nc = bacc.Bacc()
    xt = nc.declare_dram_parameter("xt", [DIM, T], BF16, isOutput=False)
    wqkvt = nc.declare_dram_parameter("wqkvt", [DIM, 3 * HG], BF16, isOutput=False)
    woutt = nc.declare_dram_parameter("woutt", [HG, DIM], BF16, isOutput=False)
    bandm = nc.declare_dram_parameter("bandm", [128, 256], BF16, isOutput=False)
    blkones = nc.declare_dram_parameter("blkones", [2, 128], BF16, isOutput=False)
    yt = nc.declare_dram_parameter("yt", [DIM, T], F32, isOutput=True)

    with tile.TileContext(nc) as tc:
        _emit(nc, tc, xt, wqkvt, woutt, bandm, blkones, yt)
    nc.finalize()
    return nc


def _emit(nc, tc, xt, wqkvt, woutt, bandm, blkones, yt):
    ctx = contextlib.ExitStack()
    with ctx:
        singles = ctx.enter_context(tc.tile_pool(name="singles", bufs=1))
        xpool = ctx.enter_context(tc.tile_pool(name="xpool", bufs=32))
        epool = ctx.enter_context(tc.tile_pool(name="epool", bufs=5))
        apool = ctx.enter_context(tc.tile_pool(name="apool", bufs=2))
        spool = ctx.enter_context(tc.tile_pool(name="spool", bufs=2))
        # PSUM budget (8 banks of 2KB/partition):
        #   pair [128,1024] bufs=2 -> 4 banks (scores, double-buffered)
        #   ot   [65,512]   bufs=3 -> 3 banks (otA/otB rotate with 1 slack)
        #   qkv  [128,512]  bufs=1 -> 1 bank (stage1/stage3/recip-broadcast)
        psq = ctx.enter_context(tc.tile_pool(name="psq", bufs=2, space="PSUM"))
        psot = ctx.enter_context(tc.tile_pool(name="psot", bufs=3, space="PSUM"))
        psmm = ctx.enter_context(tc.tile_pool(name="psmm", bufs=1, space="PSUM"))

        # ---- input DMAs, in priority order for the first matmuls ----
        xts = {}

        def emit_x(c):
            for k in range(8):
                t = xpool.tile([128, 512], BF16, tag="xt", name=f"x{c}_{k}")
                nc.sync.dma_start(out=t, in_=xt[k * 128:(k + 1) * 128,
                                                c * 512:(c + 1) * 512])
                xts[(c, k)] = t

        wq = []
        for k in range(8):
            t = xpool.tile([128, 512], BF16, tag="xt", name=f"x0_{k}")
            nc.sync.dma_start(out=t, in_=xt[k * 128:(k + 1) * 128, 0:512])
            xts[(0, k)] = t
            w = singles.tile([128, 3 * HG], BF16, name=f"wq{k}")
            nc.sync.dma_start(out=w[:, 0:HG], in_=wqkvt[k * 128:(k + 1) * 128, 0:HG])
            wq.append(w)
        for k in range(8):
            nc.sync.dma_start(out=wq[k][:, HG:2 * HG],
                              in_=wqkvt[k * 128:(k + 1) * 128, HG:2 * HG])
        for k in range(8):
            nc.sync.dma_start(out=wq[k][:, 2 * HG:3 * HG],
                              in_=wqkvt[k * 128:(k + 1) * 128, 2 * HG:3 * HG])
        band = singles.tile([128, 256], BF16, name="band")
        nc.sync.dma_start(out=band, in_=bandm[:, :])
        blkA = singles.tile([1, 128], BF16, name="blkA")
        nc.sync.dma_start(out=blkA, in_=blkones[0:1, :])
        blkB = singles.tile([1, 128], BF16, name="blkB")
        nc.sync.dma_start(out=blkB, in_=blkones[1:2, :])
        emit_x(1)
        wo = []
        for k in range(4):
            w = singles.tile([128, DIM], BF16, name=f"wo{k}")
            nc.sync.dma_start(out=w, in_=woutt[k * 128:(k + 1) * 128, :])
            wo.append(w)

        # ---- persistent SBUF tensors + on-device init ----
        qt = [singles.tile([128, T], BF16, name=f"qt{m}") for m in range(4)]
        kt = [singles.tile([128, T], BF16, name=f"kt{m}") for m in range(4)]
        vt = [singles.tile([128, HEADS_PER_CORE * 65], BF16, name=f"vt{t}")
              for t in range(16)]
        for t in range(16):
            v3 = vt[t].rearrange("p (h d) -> p h d", h=HEADS_PER_CORE)
            nc.vector.memset(v3[:, :, 64:65], 1.0)
        ediag = [[singles.tile([128, 1024], BF16, name=f"ed{j}_{i}")
                  for i in range(2)] for j in range(4)]
        for j in range(4):
            for i in range(2):
                nc.vector.memset(ediag[j][i], 0.0)
        band2 = band.rearrange("p (g w) -> p g w", g=2)

        # ---- work-unit emitters ----
        def s1_qk(c, which, m):
            cs = slice(c * 512, (c + 1) * 512)
            dst = qt if which == 0 else kt
            ps = psmm.tile([128, 512], F32, tag="qkv", name=f"p{c}_{which}_{m}")
            for k in range(8):
                nc.tensor.matmul(
                    ps,
                    lhsT=wq[k][:, which * HG + m * 128: which * HG + (m + 1) * 128],
                    rhs=xts[(c, k)],
                    start=(k == 0), stop=(k == 7),
                )
            nc.vector.tensor_copy(dst[m][:, cs], ps)

        def s1_v(c, i):
            t = c * 4 + i
            ps = psmm.tile([128, 512], F32, tag="qkv", name=f"pv{t}")
            for k in range(8):
                nc.tensor.matmul(
                    ps,
                    lhsT=xts[(c, k)][:, i * 128:(i + 1) * 128],
                    rhs=wq[k][:, 2 * HG:3 * HG],
                    start=(k == 0), stop=(k == 7),
                )
            v3 = vt[t].rearrange("p (h d) -> p h d", h=HEADS_PER_CORE)
            nc.vector.tensor_copy(
                v3[:, :, 0:64],
                ps.rearrange("p (h d) -> p h d", h=HEADS_PER_CORE))

        def s3_group(c, od, ep=False):
            cs = slice(c * 512, (c + 1) * 512)
            if ep and od % 2 == 1:
                psw = psq.tile([128, 1024], F32, tag="pair", name=f"pyq{c}_{od}")
                ps = psw[:, 0:512]
            else:
                ps = psmm.tile([128, 512], F32, tag="qkv", name=f"py{c}_{od}")
            for k in range(4):
                nc.tensor.matmul(
                    ps,
                    lhsT=wo[k][:, od * 128:(od + 1) * 128],
                    rhs=aot[(c, k)],
                    start=(k == 0), stop=(k == 3),
                )
            ys = spool.tile([128, 512], F32, tag="ys", bufs=2, name=f"ys{c}_{od}")
            if ep and od % 2 == 0:
                nc.scalar.activation(ys, ps, mybir.ActivationFunctionType.Copy)
            else:
                nc.vector.tensor_copy(ys, ps)
            nc.sync.dma_start(out=yt[od * 128:(od + 1) * 128, cs], in_=ys)

        aot = {}      # (c, hp) -> sbuf tile holding both heads, divided
        ots = {}      # (c, hp) -> (otA, otB) psum tiles
        ediag_rr = [0] * 4

        def scores_part(c, hp, tk, n_kt):
            # diag ktile j: q-columns below 128j are fully causally killed --
            # skip them in the matmul (exp already skips them; the e rect
            # stays zero from the one-time memset).
            off = 128 * (tk - (n_kt - 4)) if tk >= n_kt - 4 else 0
            q = psq.tile([128, 1024], F32, tag="pair", name=f"s{c}_{hp}_{tk}")
            for i in range(2):
                ho = i * 64
                nc.tensor.matmul(
                    q[:, i * 512 + off:(i + 1) * 512],
                    lhsT=kt[hp][ho:ho + 64, tk * 128:(tk + 1) * 128],
                    rhs=qt[hp][ho:ho + 64, c * 512 + off:(c + 1) * 512],
                    start=True, stop=True,
                    tile_position=(ho, 0),
                )
            return q

        def act_part(c, hp, tk, n_kt, q):
            diag = tk >= n_kt - 4
            if diag:
                j = tk - (n_kt - 4)
                e = ediag[j][ediag_rr[j]]
                ediag_rr[j] ^= 1
                off = 128 * j
                q3 = q.rearrange("p (g w) -> p g w", g=2)
                e3 = e.rearrange("p (g w) -> p g w", g=2)
                nc.scalar.activation(
                    e3[:, :, off:512], q3[:, :, off:512],
                    mybir.ActivationFunctionType.Exp, scale=float(SCALE))
                return (e, off)
            e = epool.tile([128, 1024], BF16, tag="e", name=f"e{c}_{hp}_{tk}")
            nc.scalar.activation(e, q, mybir.ActivationFunctionType.Exp,
                                 scale=float(SCALE))
            return (e, None)

        def band_part(eo):
            e, off = eo
            if off is None:
                return e
            e3 = e.rearrange("p (g w) -> p g w", g=2)
            nc.vector.tensor_mul(
                e3[:, :, off:off + 128], e3[:, :, off:off + 128], band2)
            return e

        def emit_pv(c, hp, tk, n_kt, e):
            if tk == 0:
                otA = psot.tile([65, 512], F32, tag="ot", name=f"oA{c}_{hp}")
                otB = psot.tile([65, 512], F32, tag="ot", name=f"oB{c}_{hp}")
                ots[(c, hp)] = (otA, otB)
            otA, otB = ots[(c, hp)]
            # tk==0 always has off==0, so every output column is initialized
            # by the start=True matmul; diag tiles only touch live columns.
            off = 128 * (tk - (n_kt - 4)) if tk >= n_kt - 4 else 0
            for i, ot in ((0, otA), (1, otB)):
                h = 2 * hp + i
                nc.tensor.matmul(
                    ot[:, off:512],
                    lhsT=vt[tk][:, h * 65:h * 65 + 65],
                    rhs=e[:, i * 512 + off:(i + 1) * 512],
                    start=(tk == 0), stop=(tk == n_kt - 1),
                )

        def div_part1(c, hp):
            # cast the two denominator rows (PSUM) to bf16 SBUF; the
            # reciprocal happens after the PE broadcast, on SBUF data.
            otA, otB = ots[(c, hp)]
            rbA = spool.tile([1, 512], BF16, tag="rbA", name=f"ba{c}_{hp}")
            rbB = spool.tile([1, 512], BF16, tag="rbB", name=f"bb2{c}_{hp}")
            nc.vector.tensor_copy(rbA, otA[64:65])
            nc.vector.tensor_copy(rbB, otB[64:65])
            return (rbA, rbB)

        def div_part2(c, hp, rec2b):
            rbA, rbB = rec2b
            otA, otB = ots.pop((c, hp))
            bcs = psmm.tile([128, 512], F32, tag="qkv", name=f"bc{c}_{hp}")
            nc.tensor.matmul(bcs, lhsT=blkA, rhs=rbA, start=True, stop=False)
            nc.tensor.matmul(bcs, lhsT=blkB, rhs=rbB, start=False, stop=True)
            bcf = spool.tile([128, 512], F32, tag="bcf", name=f"bf{c}_{hp}")
            nc.vector.tensor_copy(bcf, bcs)
            bci = spool.tile([128, 512], F32, tag="bci", name=f"bi{c}_{hp}")
            nc.vector.reciprocal_approx_fast(bci, bcf)
            a = apool.tile([128, 512], BF16, tag=f"aot{hp}", name=f"a{c}_{hp}")
            aot[(c, hp)] = a
            nc.vector.tensor_mul(a[0:64, :], otA[0:64, :], bci[0:64, :])
            nc.vector.tensor_mul(a[64:128, :], otB[0:64, :], bci[64:128, :])

        # ---- globally interleaved schedule ----
        pending = deque()   # (c, hp, tk, n_kt, e) awaiting P@V emission
        div2_q = deque()    # (c, hp, rec2b) awaiting broadcast+multiply

        def flush_one_pv():
            c_, hp_, tk_, nkt_, e_ = pending.popleft()
            emit_pv(c_, hp_, tk_, nkt_, e_)
            if tk_ == nkt_ - 1:
                div2_q.append((c_, hp_, div_part1(c_, hp_)))

        # stage1 chunk0 prologue: just enough for the first attention items
        for g in (lambda: s1_qk(0, 0, 0), lambda: s1_qk(0, 1, 0),
                  lambda: s1_v(0, 0)):
            g()

        fillers = {
            0: [lambda: s1_v(0, 1), lambda: s1_v(0, 2), lambda: s1_v(0, 3),
                lambda: s1_qk(0, 0, 1), lambda: s1_qk(0, 1, 1),
                lambda: s1_qk(0, 0, 2), lambda: s1_qk(0, 1, 2),
                lambda: s1_qk(0, 0, 3), lambda: s1_qk(0, 1, 3)]
               + [lambda w=w, m=m: s1_qk(1, w, m) for m in range(4)
                  for w in range(2)]
               + [lambda i=i: s1_v(1, i) for i in range(4)],
            1: [lambda w=w, m=m: s1_qk(2, w, m) for m in range(4)
                for w in range(2)]
               + [lambda i=i: s1_v(2, i) for i in range(4)]
               + [lambda od=od: s3_group(0, od) for od in range(8)],
            2: [lambda w=w, m=m: s1_qk(3, w, m) for m in range(4)
                for w in range(2)]
               + [lambda i=i: s1_v(3, i) for i in range(4)]
               + [lambda od=od: s3_group(1, od) for od in range(8)],
            3: [lambda od=od: s3_group(2, od) for od in range(8)],
        }

        for c in range(NCH):
            if c + 2 < NCH:
                emit_x(c + 2)
            n_kt = KT_PER_CH * (c + 1)
            todo = fillers[c]
            n_items = 4 * n_kt
            acc, done = 0.0, 0
            rate = len(todo) / n_items
            for hp in range(4):
                for u in range(n_kt // 2):
                    tka, tkb = 2 * u, 2 * u + 1
                    qa = scores_part(c, hp, tka, n_kt)
                    qb = scores_part(c, hp, tkb, n_kt)
                    eoa = act_part(c, hp, tka, n_kt, qa)
                    eob = act_part(c, hp, tkb, n_kt, qb)
                    if div2_q:
                        div_part2(*div2_q.popleft())
                    while len(pending) > PV_LAG - 2:
                        flush_one_pv()
                    pending.append((c, hp, tka, n_kt, band_part(eoa)))
                    pending.append((c, hp, tkb, n_kt, band_part(eob)))
                    acc += 2 * rate
                    while acc >= 1.0 and done < len(todo):
                        todo[done]()
                        done += 1
                        acc -= 1.0
            while done < len(todo):
                todo[done]()
                done += 1

        # epilogue: drain attention pipeline, then last output projection
        while pending or div2_q:
            if div2_q:
                div_part2(*div2_q.popleft())
            if pending:
                flush_one_pv()
        for od in range(8):
            s3_group(3, od, ep=True)


_NC_CACHE = None


def _get_nc():
    global _NC_CACHE
    if _NC_CACHE is None:
        _NC_CACHE = build_bass()
    return _NC_CACHE


def make_consts():
    k = np.arange(128)[:, None]
    t = np.arange(128)[None, :]
    tri = (t >= k).astype(np.float32)          # keep iff q-local >= k
    bandm = np.concatenate([tri, tri], axis=1)  # [128, 256]
    blk = np.zeros((2, 128), dtype=np.float32)
    blk[0, 0:64] = 1.0
    blk[1, 64:128] = 1.0
    return (bandm.astype(ml_dtypes.bfloat16), blk.astype(ml_dtypes.bfloat16))


def make_in_maps(x, w_qkv, w_out):
    x = np.asarray(x, dtype=np.float32)
    w_qkv = np.asarray(w_qkv, dtype=np.float32)
    w_out = np.asarray(w_out, dtype=np.float32)
    bandm, blk = make_consts()
    in_maps = []
    for c in range(N_CORES):
        b, g = c // 2, c % 2
        gs = slice(g * HG, (g + 1) * HG)
        wsel = np.concatenate(
            [w_qkv[0 * INNER:][gs], w_qkv[1 * INNER:][gs], w_qkv[2 * INNER:][gs]],
            axis=0)                               # [1536, 1024]
        in_maps.append({
            "xt": np.ascontiguousarray(x[b].T).astype(ml_dtypes.bfloat16),
            "wqkvt": np.ascontiguousarray(wsel.T).astype(ml_dtypes.bfloat16),
            "woutt": np.ascontiguousarray(w_out[:, gs].T).astype(ml_dtypes.bfloat16),
            "bandm": bandm,
            "blkones": blk,
        })
    return in_maps


def kernel(x, mask, w_qkv, w_out, **_):
    nc = _get_nc()
    in_maps = make_in_maps(x, w_qkv, w_out)
    res = run_bass_kernel_spmd(nc, in_maps, core_ids=list(range(N_CORES)))
    y = np.zeros((B, T, DIM), dtype=np.float32)
    for c in range(N_CORES):
        y[c // 2] += res.results[c]["yt"].T
    return y


# revision 33
# speedup vs baseline: 1.0105x; 1.0105x over previous
"""Causal self-attention Trainium2 Bass kernel.

Problem: B=4, T=2048, DIM=1024, H=16 heads, head_dim=64 (fp32).
  qkv = x @ w_qkv.T ; per-head causal softmax(q k^T / 8) v ; out @ w_out.T

Sharding (8 cores): core c -> (batch b = c//2, head-group g = c%2 of 8 heads).
Each core computes a partial output y_partial = attn_out_g @ w_out[:, g]^T
for its batch; host sums the two head-group partials per batch.

Device layout (per core):
  xt      [1024, 2048] bf16 : x[b]^T (dim-major)           -- host-transposed
  wqkvt   [1024, 1536] bf16 : [Wq|Wk|Wv]^T slice           -- host-transposed
  woutt   [ 512, 1024] bf16 : w_out[:, g]^T                -- host-transposed
  bandm   [ 128,  256] bf16 : keep-triangle (t>=k), two copies side by side
  blkones [   2,  128] bf16 : rows select head A/B cols (recip broadcast)
  yt      [1024, 2048] f32  : partial output, transposed

Structure: the whole kernel is ONE globally interleaved instruction stream.
Scores for 2 heads are row-packed (tile_position 0/64) so each (head-pair,
ktile) is one 512-col PE slot; exp runs on ScalarE; P@V accumulates per-head
output plus a ones-column denominator row.  Because exp (~1.1us/tile) is
slower than the PE slots that feed/consume it, attention alone would stall
the in-order PE queue.  So:
  - P@V emission lags its (scores, exp) item by 3 items, and the softmax
    division is split (DVE reciprocal part / PE-broadcast part) and woven
    between pv flushes so the PE never sits on a not-yet-ready dependency.
  - QKV-projection groups of the NEXT chunk and output-projection groups of
    the PREVIOUS chunk are emitted as fillers between attention items, so
    both PE and ScalarE stay busy through the whole kernel.
Causal masking: only the 4 diagonal ktiles per (head-pair, chunk) need it.
Each diagonal index j has dedicated pre-zeroed e-buffers: exp is only run on
the live columns [128j:512) of each head half (strided 2-group ACT), the
partition-dependent 128-col band is masked with one [128,(2,128)] DVE
multiply, and the fully-killed rectangle stays zero from a one-time memset.
The denominator reciprocal is broadcast across partitions with a tiny
(blkones) matmul instead of a DRAM round-trip.  Stage-3 output tiles are
DMAed PSUM->DRAM directly.
"""

import contextlib
from collections import deque

import numpy as np
import ml_dtypes

import concourse.bass as bass
import concourse.mybir as mybir
import concourse.tile as tile
from concourse import bacc
from concourse.bass_utils import run_bass_kernel_spmd

B, T, DIM = 4, 2048, 1024
NUM_HEADS, HEAD_DIM = 16, 64
INNER = NUM_HEADS * HEAD_DIM
SCALE = HEAD_DIM ** -0.5

N_CORES = 8
HEADS_PER_CORE = 8
HG = HEADS_PER_CORE * HEAD_DIM  # 512 = inner slice per core
NCH = T // 512                  # 4 token chunks
KT_PER_CH = 4                   # 128-ktok tiles per 512 chunk
PV_LAG = 6                      # items between (scores,exp) and its P@V

F32 = mybir.dt.float32
BF16 = mybir.dt.bfloat16


def build_bass():
    nc = bacc.Bacc()
    xt = nc.declare_dram_parameter("xt", [DIM, T], BF16, isOutput=False)
    wqkvt = nc.declare_dram_parameter("wqkvt", [DIM, 3 * HG], BF16, isOutput=False)
    woutt = nc.declare_dram_parameter("woutt", [HG, DIM], BF16, isOutput=False)
    bandm = nc.declare_dram_parameter("bandm", [128, 256], BF16, isOutput=False)
    blkones = nc.declare_dram_parameter("blkones", [2, 128], BF16, isOutput=False)
    yt = nc.declare_dram_parameter("yt", [DIM, T], F32, isOutput=True)

    with tile.TileContext(nc) as tc:
        _emit(nc, tc, xt, wqkvt, woutt, bandm, blkones, yt)
    nc.finalize()
    return nc


def _emit(nc, tc, xt, wqkvt, woutt, bandm, blkones, yt):
    ctx = contextlib.ExitStack()
    with ctx:
        singles = ctx.enter_context(tc.tile_pool(name="singles", bufs=1))
        xpool = ctx.enter_context(tc.tile_pool(name="xpool", bufs=32))
        epool = ctx.enter_context(tc.tile_pool(name="epool", bufs=7))
        apool = ctx.enter_context(tc.tile_pool(name="apool", bufs=2))
        spool = ctx.enter_context(tc.tile_pool(name="spool", bufs=2))
        # PSUM budget (8 banks of 2KB/partition):
        #   pair [128,1024] bufs=2 -> 4 banks (scores, double-buffered)
        #   ot   [65,512]   bufs=3 -> 3 banks (otA/otB rotate with 1 slack)
        #   qkv  [128,512]  bufs=1 -> 1 bank (stage1/stage3/recip-broadcast)
        psq = ctx.enter_context(tc.tile_pool(name="psq", bufs=2, space="PSUM"))
        psot = ctx.enter_context(tc.tile_pool(name="psot", bufs=3, space="PSUM"))
        psmm = ctx.enter_context(tc.tile_pool(name="psmm", bufs=1, space="PSUM"))

        # ---- input DMAs, in priority order for the first matmuls ----
        xts = {}

        def emit_x(c):
            for k in range(8):
                t = xpool.tile([128, 512], BF16, tag="xt", name=f"x{c}_{k}")
                nc.sync.dma_start(out=t, in_=xt[k * 128:(k + 1) * 128,
                                                c * 512:(c + 1) * 512])
                xts[(c, k)] = t

        wq = []
        for k in range(8):
            t = xpool.tile([128, 512], BF16, tag="xt", name=f"x0_{k}")
            nc.sync.dma_start(out=t, in_=xt[k * 128:(k + 1) * 128, 0:512])
            xts[(0, k)] = t
            w = singles.tile([128, 3 * HG], BF16, name=f"wq{k}")
            nc.sync.dma_start(out=w[:, 0:HG], in_=wqkvt[k * 128:(k + 1) * 128, 0:HG])
            wq.append(w)
        for k in range(8):
            nc.sync.dma_start(out=wq[k][:, HG:2 * HG],
                              in_=wqkvt[k * 128:(k + 1) * 128, HG:2 * HG])
        for k in range(8):
            nc.sync.dma_start(out=wq[k][:, 2 * HG:3 * HG],
                              in_=wqkvt[k * 128:(k + 1) * 128, 2 * HG:3 * HG])
        band = singles.tile([128, 256], BF16, name="band")
        nc.sync.dma_start(out=band, in_=bandm[:, :])
        blkA = singles.tile([1, 128], BF16, name="blkA")
        nc.sync.dma_start(out=blkA, in_=blkones[0:1, :])
        blkB = singles.tile([1, 128], BF16, name="blkB")
        nc.sync.dma_start(out=blkB, in_=blkones[1:2, :])
        wo = []
        for k in range(4):
            w = singles.tile([128, DIM], BF16, name=f"wo{k}")
            nc.sync.dma_start(out=w, in_=woutt[k * 128:(k + 1) * 128, :])
            wo.append(w)
        emit_x(1)

        # ---- persistent SBUF tensors + on-device init ----
        qt = [singles.tile([128, T], BF16, name=f"qt{m}") for m in range(4)]
        kt = [singles.tile([128, T], BF16, name=f"kt{m}") for m in range(4)]
        vt = [singles.tile([128, HEADS_PER_CORE * 65], BF16, name=f"vt{t}")
              for t in range(16)]
        for t in range(16):
            v3 = vt[t].rearrange("p (h d) -> p h d", h=HEADS_PER_CORE)
            nc.vector.memset(v3[:, :, 64:65], 1.0)
        ediag = [[singles.tile([128, 1024], BF16, name=f"ed{j}_{i}")
                  for i in range(2)] for j in range(4)]
        for j in range(4):
            for i in range(2):
                nc.vector.memset(ediag[j][i], 0.0)
        band2 = band.rearrange("p (g w) -> p g w", g=2)

        # ---- work-unit emitters ----
        def s1_qk(c, which, m):
            cs = slice(c * 512, (c + 1) * 512)
            dst = qt if which == 0 else kt
            ps = psmm.tile([128, 512], F32, tag="qkv", name=f"p{c}_{which}_{m}")
            for k in range(8):
                nc.tensor.matmul(
                    ps,
                    lhsT=wq[k][:, which * HG + m * 128: which * HG + (m + 1) * 128],
                    rhs=xts[(c, k)],
                    start=(k == 0), stop=(k == 7),
                )
            nc.vector.tensor_copy(dst[m][:, cs], ps)

        def s1_v(c, i):
            t = c * 4 + i
            ps = psmm.tile([128, 512], F32, tag="qkv", name=f"pv{t}")
            for k in range(8):
                nc.tensor.matmul(
                    ps,
                    lhsT=xts[(c, k)][:, i * 128:(i + 1) * 128],
                    rhs=wq[k][:, 2 * HG:3 * HG],
                    start=(k == 0), stop=(k == 7),
                )
            v3 = vt[t].rearrange("p (h d) -> p h d", h=HEADS_PER_CORE)
            nc.vector.tensor_copy(
                v3[:, :, 0:64],
                ps.rearrange("p (h d) -> p h d", h=HEADS_PER_CORE))

        def s3_group(c, od, ep=False):
            cs = slice(c * 512, (c + 1) * 512)
            if ep and od % 2 == 1:
                psw = psq.tile([128, 1024], F32, tag="pair", name=f"pyq{c}_{od}")
                ps = psw[:, 0:512]
            else:
                ps = psmm.tile([128, 512], F32, tag="qkv", name=f"py{c}_{od}")
            for k in range(4):
                nc.tensor.matmul(
                    ps,
                    lhsT=wo[k][:, od * 128:(od + 1) * 128],
                    rhs=aot[(c, k)],
                    start=(k == 0), stop=(k == 3),
                )
            ys = spool.tile([128, 512], F32, tag="ys", bufs=2, name=f"ys{c}_{od}")
            nc.vector.tensor_copy(ys, ps)
            nc.sync.dma_start(out=yt[od * 128:(od + 1) * 128, cs], in_=ys)

        aot = {}      # (c, hp) -> sbuf tile holding both heads, divided
        ots = {}      # (c, hp) -> (otA, otB) psum tiles
        ediag_rr = [0] * 4

        def scores_part(c, hp, tk, n_kt):
            # diag ktile j: q-columns below 128j are fully causally killed --
            # skip them in the matmul (exp already skips them; the e rect
            # stays zero from the one-time memset).
            off = 128 * (tk - (n_kt - 4)) if tk >= n_kt - 4 else 0
            q = psq.tile([128, 1024], F32, tag="pair", name=f"s{c}_{hp}_{tk}")
            for i in range(2):
                ho = i * 64
                nc.tensor.matmul(
                    q[:, i * 512 + off:(i + 1) * 512],
                    lhsT=kt[hp][ho:ho + 64, tk * 128:(tk + 1) * 128],
                    rhs=qt[hp][ho:ho + 64, c * 512 + off:(c + 1) * 512],
                    start=True, stop=True,
                    tile_position=(ho, 0),
                )
            return q

        def act_part(c, hp, tk, n_kt, q):
            diag = tk >= n_kt - 4
            if diag:
                j = tk - (n_kt - 4)
                e = ediag[j][ediag_rr[j]]
                ediag_rr[j] ^= 1
                off = 128 * j
                q3 = q.rearrange("p (g w) -> p g w", g=2)
                e3 = e.rearrange("p (g w) -> p g w", g=2)
                nc.scalar.activation(
                    e3[:, :, off:512], q3[:, :, off:512],
                    mybir.ActivationFunctionType.Exp, scale=float(SCALE))
                return (e, off)
            e = epool.tile([128, 1024], BF16, tag="e", name=f"e{c}_{hp}_{tk}")
            nc.scalar.activation(e, q, mybir.ActivationFunctionType.Exp,
                                 scale=float(SCALE))
            return (e, None)

        def band_part(eo):
            e, off = eo
            if off is None:
                return e
            e3 = e.rearrange("p (g w) -> p g w", g=2)
            nc.vector.tensor_mul(
                e3[:, :, off:off + 128], e3[:, :, off:off + 128], band2)
            return e

        def emit_pv(c, hp, tk, n_kt, e):
            if tk == 0:
                otA = psot.tile([65, 512], F32, tag="ot", name=f"oA{c}_{hp}")
                otB = psot.tile([65, 512], F32, tag="ot", name=f"oB{c}_{hp}")
                ots[(c, hp)] = (otA, otB)
            otA, otB = ots[(c, hp)]
            # tk==0 always has off==0, so every output column is initialized
            # by the start=True matmul; diag tiles only touch live columns.
            off = 128 * (tk - (n_kt - 4)) if tk >= n_kt - 4 else 0
            for i, ot in ((0, otA), (1, otB)):
                h = 2 * hp + i
                nc.tensor.matmul(
                    ot[:, off:512],
                    lhsT=vt[tk][:, h * 65:h * 65 + 65],
                    rhs=e[:, i * 512 + off:(i + 1) * 512],
                    start=(tk == 0), stop=(tk == n_kt - 1),
                )

        def div_part1(c, hp):
            # cast the two denominator rows (PSUM) to bf16 SBUF; the
            # reciprocal happens after the PE broadcast, on SBUF data.
            otA, otB = ots[(c, hp)]
            rbA = spool.tile([1, 512], BF16, tag="rbA", name=f"ba{c}_{hp}")
            rbB = spool.tile([1, 512], BF16, tag="rbB", name=f"bb2{c}_{hp}")
            nc.vector.tensor_copy(rbA, otA[64:65])
            nc.vector.tensor_copy(rbB, otB[64:65])
            return (rbA, rbB)

        def div_part2(c, hp, rec2b):
            rbA, rbB = rec2b
            otA, otB = ots.pop((c, hp))
            bcs = psmm.tile([128, 512], F32, tag="qkv", name=f"bc{c}_{hp}")
            nc.tensor.matmul(bcs, lhsT=blkA, rhs=rbA, start=True, stop=False)
            nc.tensor.matmul(bcs, lhsT=blkB, rhs=rbB, start=False, stop=True)
            bcf = spool.tile([128, 512], F32, tag="bcf", name=f"bf{c}_{hp}")
            nc.vector.tensor_copy(bcf, bcs)
            bci = spool.tile([128, 512], F32, tag="bci", name=f"bi{c}_{hp}")
            nc.vector.reciprocal_approx_fast(bci, bcf)
            a = apool.tile([128, 512], BF16, tag=f"aot{hp}", name=f"a{c}_{hp}")
            aot[(c, hp)] = a
            nc.vector.tensor_mul(a[0:64, :], otA[0:64, :], bci[0:64, :])
            nc.vector.tensor_mul(a[64:128, :], otB[0:64, :], bci[64:128, :])

        # ---- globally interleaved schedule ----
        pending = deque()   # (c, hp, tk, n_kt, e) awaiting P@V emission
        div2_q = deque()    # (c, hp, rec2b) awaiting broadcast+multiply

        def flush_one_pv():
            c_, hp_, tk_, nkt_, e_ = pending.popleft()
            emit_pv(c_, hp_, tk_, nkt_, e_)
            if tk_ == nkt_ - 1:
                div2_q.append((c_, hp_, div_part1(c_, hp_)))

        # stage1 chunk0 prologue: just enough for the first attention items
        for g in (lambda: s1_qk(0, 0, 0), lambda: s1_qk(0, 1, 0),
                  lambda: s1_v(0, 0)):
            g()

        fillers = {
            0: [lambda: s1_v(0, 1), lambda: s1_v(0, 2), lambda: s1_v(0, 3),
                lambda: s1_qk(0, 0, 1), lambda: s1_qk(0, 1, 1),
                lambda: s1_qk(0, 0, 2), lambda: s1_qk(0, 1, 2),
                lambda: s1_qk(0, 0, 3), lambda: s1_qk(0, 1, 3)]
               + [lambda w=w, m=m: s1_qk(1, w, m) for m in range(4)
                  for w in range(2)]
               + [lambda i=i: s1_v(1, i) for i in range(4)],
            1: [lambda w=w, m=m: s1_qk(2, w, m) for m in range(4)
                for w in range(2)]
               + [lambda i=i: s1_v(2, i) for i in range(4)]
               + [lambda od=od: s3_group(0, od) for od in range(8)],
            2: [lambda w=w, m=m: s1_qk(3, w, m) for m in range(4)
                for w in range(2)]
               + [lambda i=i: s1_v(3, i) for i in range(4)]
               + [lambda od=od: s3_group(1, od) for od in range(8)],
            3: [lambda od=od: s3_group(2, od) for od in range(8)],
        }

        for c in range(NCH):
            if c + 2 < NCH:
                emit_x(c + 2)
            n_kt = KT_PER_CH * (c + 1)
            todo = fillers[c]
            n_items = 4 * n_kt
            acc, done = 0.0, 0
            rate = len(todo) / n_items
            for hp in range(4):
                for u in range(n_kt // 2):
                    tka, tkb = 2 * u, 2 * u + 1
                    qa = scores_part(c, hp, tka, n_kt)
                    qb = scores_part(c, hp, tkb, n_kt)
                    eoa = act_part(c, hp, tka, n_kt, qa)
                    eob = act_part(c, hp, tkb, n_kt, qb)
                    if div2_q:
                        div_part2(*div2_q.popleft())
                    while len(pending) > PV_LAG - 2:
                        flush_one_pv()
                    pending.append((c, hp, tka, n_kt, band_part(eoa)))
                    pending.append((c, hp, tkb, n_kt, band_part(eob)))
                    acc += 2 * rate
                    while acc >= 1.0 and done < len(todo):
                        todo[done]()
                        done += 1
                        acc -= 1.0
            while done < len(todo):
                todo[done]()
                done += 1

        # epilogue: drain attention pipeline, then last output projection
        while pending or div2_q:
            if div2_q:
                div_part2(*div2_q.popleft())
            if pending:
                flush_one_pv()
        for od in range(8):
            s3_group(3, od, ep=True)


_NC_CACHE = None


def _get_nc():
    global _NC_CACHE
    if _NC_CACHE is None:
        _NC_CACHE = build_bass()
    return _NC_CACHE


def make_consts():
    k = np.arange(128)[:, None]
    t = np.arange(128)[None, :]
    tri = (t >= k).astype(np.float32)          # keep iff q-local >= k
    bandm = np.concatenate([tri, tri], axis=1)  # [128, 256]
    blk = np.zeros((2, 128), dtype=np.float32)
    blk[0, 0:64] = 1.0
    blk[1, 64:128] = 1.0
    return (bandm.astype(ml_dtypes.bfloat16), blk.astype(ml_dtypes.bfloat16))


def make_in_maps(x, w_qkv, w_out):
    x = np.asarray(x, dtype=np.float32)
    w_qkv = np.asarray(w_qkv, dtype=np.float32)
    w_out = np.asarray(w_out, dtype=np.float32)
    bandm, blk = make_consts()
    in_maps = []
    for c in range(N_CORES):
        b, g = c // 2, c % 2
        gs = slice(g * HG, (g + 1) * HG)
        wsel = np.concatenate(
            [w_qkv[0 * INNER:][gs], w_qkv[1 * INNER:][gs], w_qkv[2 * INNER:][gs]],
            axis=0)                               # [1536, 1024]
        in_maps.append({
            "xt": np.ascontiguousarray(x[b].T).astype(ml_dtypes.bfloat16),
            "wqkvt": np.ascontiguousarray(wsel.T).astype(ml_dtypes.bfloat16),
            "woutt": np.ascontiguousarray(w_out[:, gs].T).astype(ml_dtypes.bfloat16),
            "bandm": bandm,
            "blkones": blk,
        })
    return in_maps


def kernel(x, mask, w_qkv, w_out, **_):
    nc = _get_nc()
    in_maps = make_in_maps(x, w_qkv, w_out)
    res = run_bass_kernel_spmd(nc, in_maps, core_ids=list(range(N_CORES)))
    y = np.zeros((B, T, DIM), dtype=np.float32)
    for c in range(N_CORES):
        y[c // 2] += res.results[c]["yt"].T
    return y
